# revision 8
# baseline (speedup 1.0000x reference)
"""Trainium2 Bass kernel for nn_GATLayer (gnn_message_passing).

v2 deltas over the original 18794 ns baseline (sim + HW verified 18445 ns):
  * Wa = [W | a_r^col | W.T] is ONE host-concatenated const tile (pure
    layout, no host math), so the w_t chain needs no separate a_r DMA and
    w_base is built by a single matmul: stationary a_col broadcast across
    128 columns via a stride-0 free dim against W, plus one PSUM->SBUF copy
    (replaces the 4-hop wt/wrep chain and the DVE W-transposes; W.T arrives
    in the same tile).
  * The first h super-tile streams as two half-DMAs so the mult chain
    starts ~360 ns earlier against the now-early w_base.
  * Finale: 1/Z broadcast to all partitions with one reciprocal + one
    stride-0 stationary matmul; ybc = (u broadcast) @ W.T in one matmul
    (stationary u with a stride-0 free dim); the [128, 8, 64] out tile is
    materialized by DVE/ACT copies that apply the 1/Z scale.

Math: the reference computes
    Wh = h @ W.T
    e[i, j] = (Wh @ a_l)[i] + (Wh @ a_r)[j] + b
    out = softmax(e, axis=1) @ Wh
Because e[i, :] = const_i + t where t = Wh @ a_r, every softmax row equals
softmax(t) exactly (row-constant shifts cancel).  Hence
    out[i, :] = softmax(t) @ Wh = ((p @ h) @ W.T) / sum(p),  p = exp(t - c)
for any constant c, identical for all rows i.  The kernel computes the
weighted column-sum u = p @ h and Z = sum(p) in one streaming pass over h,
then projects y = (u @ W.T) / Z and broadcasts y to its row block.

Layout: h streams in 8 super-tiles of [128, 8, 64] where partition p holds 8
CONSECUTIVE rows (row = 1024*s + 8*p + j) — both DMA sides are then fully
linear (no sub-512B segments).  The u-matmul contracts row-class j across
partitions; PSUM accumulation over all (s, j) gives the same u.

Engine split: t = h@w_t runs as elementwise mult (DVE for the 5 earliest-
arriving super-tiles, Pool for the last 3) + segmented reduce (DVE);  exp on
ACT (with accum_out Z partials, except the tail tile);  u-accumulation on PE
(h_s as stationary, p column as moving);  finale on PE/DVE/ACT.

Toolchain constraint: walrus allows ONE sync-wait per instruction
(bacc legalizes the rest into EventSemaphores).  The tiny "funnel" copies
and junk prefetch matmuls absorb DMA-queue waits so every hot instruction
carries at most one; DMA queues are routed so no queue is reused.

Each of the 8 cores runs the same program and emits one [1024, 64] row block
(all blocks are equal); the host concatenates them to the full [8192, 64].
"""

import sys
from contextlib import ExitStack

sys.path.insert(0, "/opt/trn_rl_repo")

import numpy as np

import concourse.bass as bass
import concourse.mybir as mybir
import concourse.tile as tile
from concourse import bacc, bass_utils
from concourse.tile_rust import add_dep_helper

N = 8192
D = 64
NCORES = 8
ROWS_PER_CORE = N // NCORES  # 1024
P = 128                      # SBUF partitions
NT = N // P                  # 64 row tiles of h
SUP = 8                      # row tiles per super-tile
NSUP = NT // SUP             # 8 super-tiles
# t-mult engine per super-tile: DVE takes the first four (earliest DMA
# arrivals; Pool cannot start before ~h4 lands anyway and its mults cost ~2x,
# so a bigger Pool share only builds backlog behind the last arrival).
DVE_MULT_TILES = {0, 1, 2, 3, 4}
FP32 = mybir.dt.float32
EXP_SHIFT = -8.0             # exp(t - 8): cancels in u/Z, guards overflow


OUT_J = ROWS_PER_CORE // P   # 8 output rows per partition


def _bcast_out(t):
    """View a [P, D] tile as [P, OUT_J, D] via a step-0 middle dim."""
    return bass.AP(tensor=t.tensor, offset=t.offset, ap=[t.ap[0], [0, OUT_J], t.ap[1]])


def build_kernel(ctx, tc, h, Wa, out):
    nc = tc.nc
    const = ctx.enter_context(tc.tile_pool(name="const", bufs=1))
    hpool = ctx.enter_context(tc.tile_pool(name="hbuf", bufs=NSUP))
    prpool = ctx.enter_context(tc.tile_pool(name="prod", bufs=NSUP))
    spool = ctx.enter_context(tc.tile_pool(name="scratch", bufs=2))
    tppool = ctx.enter_context(tc.tile_pool(name="tp", bufs=NSUP))
    fpool = ctx.enter_context(tc.tile_pool(name="funnel", bufs=NSUP))
    ppool = ctx.enter_context(tc.tile_pool(name="psum", bufs=1, space="PSUM"))
    upool = ctx.enter_context(tc.tile_pool(name="upsum", bufs=1, space="PSUM"))

    # --- constants.  One host-concatenated tile Wa = [W | a_r^col | W.T]
    # (layout only) via SWDGE; the whole w chain is ONE matmul: stationary
    # a_col broadcast across 128 columns (stride-0 free dim) against W gives
    # w_t replicated on every partition, plus one PSUM->SBUF copy.
    Wa_sb = const.tile([D, 2 * D + 2], FP32)
    nc.gpsimd.dma_start(out=Wa_sb[:, 0 : 2 * D + 1], in_=Wa)
    Wsb = Wa_sb[:, 0:D]
    a_col = Wa_sb[:, D : D + 1]
    WT_sb = Wa_sb[:, D + 1 : 2 * D + 1]

    # scatter metadata + descriptor prep (gen early; trigger fires at the end)
    out_sb = spool.tile([P, OUT_J, D], FP32)

    junk_ps = upool.tile([1, 1], FP32)
    ones_col = const.tile([P, 1], FP32)
    nc.vector.memset(ones_col, 1.0)
    bias_sh = const.tile([P, 1], FP32)
    nc.vector.memset(bias_sh, EXP_SHIFT)

    acol_bc = bass.AP(tensor=Wa_sb.tensor, offset=a_col.offset,
                      ap=[a_col.ap[0], [0, P]])
    wb_ps = ppool.tile([P, D], FP32)
    pre_w = nc.tensor.matmul(
        junk_ps, lhsT=Wa_sb[:, 0:1], rhs=Wa_sb[:, 0:1], start=True, stop=True
    )
    wb_mm = nc.tensor.matmul(wb_ps, lhsT=acol_bc, rhs=Wsb, start=True, stop=True)
    add_dep_helper(wb_mm.ins, pre_w.ins, sync=False, reason="pe sees Wa dma first")
    w_base = const.tile([P, D], FP32)
    nc.scalar.copy(w_base, wb_ps)

    # per-super-tile partial Z sums, reduced at the end
    z_parts = const.tile([P, NSUP], FP32)
    u_ps = upool.tile([D, 1], FP32)

    # h viewed as [s][p][j][d], row = 1024*s + 8*p + j: fully linear DMA
    hview = h.rearrange("(s p j) d -> s p j d", j=SUP, p=P)

    prev_mul = {}
    for s in range(NSUP):
        # s=0 streams in two half-DMAs so the first mult starts ~a half-
        # transfer earlier; the whole DVE chain (which sets the steady-state
        # floor) slides left with it.
        nhalf = 2 if s == 0 else 1  # small first piece starts the mult chain early
        jh = SUP // nhalf
        halves = []
        for hh in range(nhalf):
            h_h = hpool.tile([P, jh, D], FP32, tag="hbuf")
            nc.sync.dma_start(out=h_h, in_=hview[s][:, hh * jh : (hh + 1) * jh, :])
            halves.append(h_h)
        # t-path: prod = h * w_t (broadcast), then segmented sum over d.
        # Order-only dep edges chain each engine's mults in arrival order.
        mul_eng = nc.vector if s in DVE_MULT_TILES else nc.gpsimd
        t_sup = tppool.tile([P, SUP], FP32)
        for hh, h_h in enumerate(halves):
            # Only each engine's FIRST mult needs a funnel: it waits on ACT
            # (w_base), with the funnel absorbing the DMA wait.  Later mults
            # have already observed the ACT tick, so their single wait slot
            # takes the DMA directly and the funnel is unnecessary.
            if mul_eng not in prev_mul:
                jd = fpool.tile([1, 1], FP32)
                fun = mul_eng.tensor_copy(jd, h_h[0:1, 0, 0:1])
                pin = fun
            else:
                fun = None
                pin = prev_mul[mul_eng]
            prod = prpool.tile([P, jh, D], FP32, tag="prod")
            wsrc = bass.AP(
                tensor=w_base.tensor, offset=w_base.offset,
                ap=[w_base.ap[0], [0, jh], w_base.ap[1]],
            )
            mul = mul_eng.tensor_mul(prod, h_h, wsrc)
            add_dep_helper(mul.ins, pin.ins, sync=False, reason="mult stream order")
            prev_mul[mul_eng] = mul
            nc.vector.tensor_reduce(
                out=t_sup[:, hh * jh : (hh + 1) * jh],
                in_=prod,
                axis=mybir.AxisListType.X,
                op=mybir.AluOpType.add,
            )
        p_sup = tppool.tile([P, SUP], FP32)
        # accum_out gives this super-tile's Z partial for free on ACT.  The
        # LAST exp sits on the critical tail, so its accumulator-read cost is
        # skipped there and the Z partial comes from a parallel DVE reduce.
        last = s == NSUP - 1
        nc.scalar.activation(
            out=p_sup,
            in_=t_sup,
            func=mybir.ActivationFunctionType.Exp,
            bias=bias_sh,
            scale=1.0,
            accum_out=None if last else z_parts[:, s : s + 1],
        )
        if last:
            nc.vector.tensor_reduce(
                out=z_parts[:, s : s + 1],
                in_=p_sup,
                axis=mybir.AxisListType.X,
                op=mybir.AluOpType.add,
            )
        # PE prefetch touching each h tile: absorbs the DMA wait so the
        # first real u-matmul on that tile carries only the ACT wait.
        pres = [
            nc.tensor.matmul(
                junk_ps, lhsT=hb[:, 0, 0:1], rhs=hb[:, 0, 0:1], start=True, stop=True
            )
            for hb in halves
        ]
        for j in range(SUP):
            k = s * SUP + j
            hb = halves[j // jh]
            mm = nc.tensor.matmul(
                u_ps,
                lhsT=hb[:, j % jh, :],
                rhs=p_sup[:, j : j + 1],
                start=(k == 0),
                stop=(k == NT - 1),
            )
            if j % jh == 0:
                add_dep_helper(
                    mm.ins, pres[j // jh].ins, sync=False, reason="pe sees h dma"
                )

    # --- finale: Z, ybc = (u/Z broadcast) @ W.T, scatter out ---
    zcol = spool.tile([P, 1], FP32)
    nc.vector.tensor_reduce(
        out=zcol, in_=z_parts, axis=mybir.AxisListType.X, op=mybir.AluOpType.add
    )
    z_ps = ppool.tile([1, 1], FP32)
    nc.tensor.matmul(z_ps, lhsT=zcol, rhs=ones_col, start=True, stop=True)
    recip = spool.tile([1, 1], FP32)
    nc.vector.reciprocal(recip, z_ps)
    # broadcast 1/Z to all partitions: stationary stride-0 against a one
    rec_bc = bass.AP(tensor=recip.tensor, offset=recip.offset,
                     ap=[recip.ap[0], [0, P]])
    rec_ps = ppool.tile([P, 1], FP32)
    nc.tensor.matmul(rec_ps, lhsT=rec_bc, rhs=ones_col[0:1, :], start=True, stop=True)
    rec_sb = spool.tile([P, 1], FP32)
    nc.vector.tensor_copy(rec_sb, rec_ps)
    u_sb = spool.tile([D, 1], FP32)
    nc.vector.tensor_copy(u_sb, u_ps)
    # ybc[p, d] = sum_k u[k] W.T[k, d]: stationary u broadcast via stride-0
    ubc = bass.AP(tensor=u_sb.tensor, offset=u_sb.offset,
                  ap=[u_sb.ap[0], [0, P]])
    ybc_ps = ppool.tile([P, D], FP32)
    nc.tensor.matmul(ybc_ps, lhsT=ubc, rhs=WT_sb, start=True, stop=True)
    # materialize [128, 8, 64] with the 1/Z scale split across DVE/ACT/Pool
    def ybc_bc(J):
        return bass.AP(tensor=ybc_ps.tensor, offset=ybc_ps.offset,
                       ap=[ybc_ps.ap[0], [0, J], ybc_ps.ap[1]])
    def rec_bc3(J):
        return bass.AP(tensor=rec_sb.tensor, offset=rec_sb.offset,
                       ap=[rec_sb.ap[0], [0, J], [0, D]])
    nc.vector.tensor_mul(out_sb[:, 0:4, :], ybc_bc(4), rec_bc3(4))
    nc.scalar.activation(out=out_sb[:, 4:8, :], in_=ybc_bc(4),
                         func=mybir.ActivationFunctionType.Copy,
                         bias=0.0, scale=rec_sb)
    nc.sync.dma_start(out=out.rearrange("(p j) d -> p j d", j=OUT_J), in_=out_sb)


def build_bass():
    nc = bacc.Bacc("TRN2", debug=False, target_bir_lowering=False)
    h = nc.dram_tensor("h", [N, D], FP32, kind="ExternalInput").ap()
    Wa = nc.dram_tensor("Wa", [D, 2 * D + 1], FP32, kind="ExternalInput").ap()
    out = nc.dram_tensor("out", [ROWS_PER_CORE, D], FP32, kind="ExternalOutput").ap()
    with tile.TileContext(nc) as tc:
        with ExitStack() as ctx:
            build_kernel(ctx, tc, h, Wa, out)
    nc.compile()
    return nc


_NC_CACHE = None


def _get_nc():
    global _NC_CACHE
    if _NC_CACHE is None:
        _NC_CACHE = build_bass()
    return _NC_CACHE


def kernel(**inputs) -> np.ndarray:
    h = np.ascontiguousarray(np.asarray(inputs["h"], dtype=np.float32))
    W = np.ascontiguousarray(np.asarray(inputs["W"], dtype=np.float32))
    aw = np.ascontiguousarray(np.asarray(inputs["attn_w"], dtype=np.float32))
    assert h.shape == (N, D) and W.shape == (D, D) and aw.shape == (1, 2 * D)

    Wa = np.ascontiguousarray(
        np.concatenate([W, aw[0, D : 2 * D][:, None], W.T], axis=1)
    )
    nc = _get_nc()
    in_map = {"h": h, "Wa": Wa}
    in_maps = [in_map for _ in range(NCORES)]
    res = bass_utils.run_bass_kernel_spmd(nc, in_maps, list(range(NCORES)))
    blocks = [res.results[i]["out"] for i in range(NCORES)]
    return np.concatenate(blocks, axis=0)


if __name__ == "__main__":
    nc = _get_nc()
    print("Bass program built OK")



# revision 9
# speedup vs baseline: 1.0290x; 1.0290x over previous
"""Trainium2 Bass kernel for nn_GATLayer (gnn_message_passing).

v2 deltas over the original 18794 ns baseline (sim + HW verified 18445 ns):
  * Wa = [W | a_r^col | W.T] is ONE host-concatenated const tile (pure
    layout, no host math), so the w_t chain needs no separate a_r DMA and
    w_base is built by a single matmul: stationary a_col broadcast across
    128 columns via a stride-0 free dim against W, plus one PSUM->SBUF copy
    (replaces the 4-hop wt/wrep chain and the DVE W-transposes; W.T arrives
    in the same tile).
  * The first h super-tile streams as two half-DMAs so the mult chain
    starts ~360 ns earlier against the now-early w_base.
  * Finale: 1/Z broadcast to all partitions with one reciprocal + one
    stride-0 stationary matmul; ybc = (u broadcast) @ W.T in one matmul
    (stationary u with a stride-0 free dim); the [128, 8, 64] out tile is
    materialized by DVE/ACT copies that apply the 1/Z scale.

Math: the reference computes
    Wh = h @ W.T
    e[i, j] = (Wh @ a_l)[i] + (Wh @ a_r)[j] + b
    out = softmax(e, axis=1) @ Wh
Because e[i, :] = const_i + t where t = Wh @ a_r, every softmax row equals
softmax(t) exactly (row-constant shifts cancel).  Hence
    out[i, :] = softmax(t) @ Wh = ((p @ h) @ W.T) / sum(p),  p = exp(t - c)
for any constant c, identical for all rows i.  The kernel computes the
weighted column-sum u = p @ h and Z = sum(p) in one streaming pass over h,
then projects y = (u @ W.T) / Z and broadcasts y to its row block.

Layout: h streams in 8 super-tiles of [128, 8, 64] where partition p holds 8
CONSECUTIVE rows (row = 1024*s + 8*p + j) — both DMA sides are then fully
linear (no sub-512B segments).  The u-matmul contracts row-class j across
partitions; PSUM accumulation over all (s, j) gives the same u.

Engine split: t = h@w_t runs as elementwise mult (DVE for the 5 earliest-
arriving super-tiles, Pool for the last 3) + segmented reduce (DVE);  exp on
ACT (with accum_out Z partials, except the tail tile);  u-accumulation on PE
(h_s as stationary, p column as moving);  finale on PE/DVE/ACT.

Toolchain constraint: walrus allows ONE sync-wait per instruction
(bacc legalizes the rest into EventSemaphores).  The tiny "funnel" copies
and junk prefetch matmuls absorb DMA-queue waits so every hot instruction
carries at most one; DMA queues are routed so no queue is reused.

Each of the 8 cores runs the same program and emits one [1024, 64] row block
(all blocks are equal); the host concatenates them to the full [8192, 64].
"""

import sys
from contextlib import ExitStack

sys.path.insert(0, "/opt/trn_rl_repo")

import numpy as np

import concourse.bass as bass
import concourse.mybir as mybir
import concourse.tile as tile
from concourse import bacc, bass_utils
from concourse.tile_rust import add_dep_helper

N = 8192
D = 64
NCORES = 8
ROWS_PER_CORE = N // NCORES  # 1024
P = 128                      # SBUF partitions
NT = N // P                  # 64 row tiles of h
SUP = 8                      # row tiles per super-tile
NSUP = NT // SUP             # 8 super-tiles
# t-mult engine per super-tile: DVE takes the first four (earliest DMA
# arrivals; Pool cannot start before ~h4 lands anyway and its mults cost ~2x,
# so a bigger Pool share only builds backlog behind the last arrival).
DVE_MULT_TILES = {0, 1, 2, 3, 4}
FP32 = mybir.dt.float32
EXP_SHIFT = -8.0             # exp(t - 8): cancels in u/Z, guards overflow


OUT_J = ROWS_PER_CORE // P   # 8 output rows per partition


def _bcast_out(t):
    """View a [P, D] tile as [P, OUT_J, D] via a step-0 middle dim."""
    return bass.AP(tensor=t.tensor, offset=t.offset, ap=[t.ap[0], [0, OUT_J], t.ap[1]])


def build_kernel(ctx, tc, h, Wa, out):
    nc = tc.nc
    const = ctx.enter_context(tc.tile_pool(name="const", bufs=1))
    hpool = ctx.enter_context(tc.tile_pool(name="hbuf", bufs=NSUP))
    prpool = ctx.enter_context(tc.tile_pool(name="prod", bufs=NSUP))
    spool = ctx.enter_context(tc.tile_pool(name="scratch", bufs=2))
    tppool = ctx.enter_context(tc.tile_pool(name="tp", bufs=NSUP))
    fpool = ctx.enter_context(tc.tile_pool(name="funnel", bufs=NSUP))
    ppool = ctx.enter_context(tc.tile_pool(name="psum", bufs=1, space="PSUM"))
    upool = ctx.enter_context(tc.tile_pool(name="upsum", bufs=1, space="PSUM"))

    # --- constants.  One host-concatenated tile Wa = [W | a_r^col | W.T]
    # (layout only) via SWDGE; the whole w chain is ONE matmul: stationary
    # a_col broadcast across 128 columns (stride-0 free dim) against W gives
    # w_t replicated on every partition, plus one PSUM->SBUF copy.
    Wa_sb = const.tile([D, 2 * D + 2], FP32)
    nc.gpsimd.dma_start(out=Wa_sb[:, 0 : 2 * D + 1], in_=Wa)
    Wsb = Wa_sb[:, 0:D]
    a_col = Wa_sb[:, D : D + 1]
    WT_sb = Wa_sb[:, D + 1 : 2 * D + 1]

    # scatter metadata + descriptor prep (gen early; trigger fires at the end)
    out_sb = spool.tile([P, OUT_J, D], FP32)

    junk_ps = upool.tile([1, 1], FP32)
    ones_col = const.tile([P, 1], FP32)
    nc.vector.memset(ones_col, 1.0)
    bias_sh = const.tile([P, 1], FP32)
    nc.vector.memset(bias_sh, EXP_SHIFT)

    acol_bc = bass.AP(tensor=Wa_sb.tensor, offset=a_col.offset,
                      ap=[a_col.ap[0], [0, P]])
    wb_ps = ppool.tile([P, D], FP32)
    pre_w = nc.tensor.matmul(
        junk_ps, lhsT=Wa_sb[:, 0:1], rhs=Wa_sb[:, 0:1], start=True, stop=True
    )
    wb_mm = nc.tensor.matmul(wb_ps, lhsT=acol_bc, rhs=Wsb, start=True, stop=True)
    add_dep_helper(wb_mm.ins, pre_w.ins, sync=False, reason="pe sees Wa dma first")
    w_base = const.tile([P, D], FP32)
    nc.scalar.copy(w_base, wb_ps)

    u_ps = upool.tile([D, 1], FP32)
    z_ps = upool.tile([D, 1], FP32)
    ones_zbc = bass.AP(tensor=ones_col.tensor, offset=ones_col.offset,
                       ap=[ones_col.ap[0], [0, D]])

    # h viewed as [s][p][j][d], row = 1024*s + 8*p + j: fully linear DMA
    hview = h.rearrange("(s p j) d -> s p j d", j=SUP, p=P)

    prev_mul = {}
    for s in range(NSUP):
        # s=0 streams in two half-DMAs so the first mult starts ~a half-
        # transfer earlier; the whole DVE chain (which sets the steady-state
        # floor) slides left with it.
        nhalf = 2 if s in (0, NSUP - 1) else 1  # split first (early start) and last (short tail)
        jh = SUP // nhalf
        halves = []
        for hh in range(nhalf):
            h_h = hpool.tile([P, jh, D], FP32, tag="hbuf")
            nc.sync.dma_start(out=h_h, in_=hview[s][:, hh * jh : (hh + 1) * jh, :])
            halves.append(h_h)
        # t-path: prod = h * w_t (broadcast), then segmented sum over d.
        # Order-only dep edges chain each engine's mults in arrival order.
        mul_eng = nc.vector if s in DVE_MULT_TILES else nc.gpsimd
        t_sup = tppool.tile([P, SUP], FP32)
        for hh, h_h in enumerate(halves):
            # Only each engine's FIRST mult needs a funnel: it waits on ACT
            # (w_base), with the funnel absorbing the DMA wait.  Later mults
            # have already observed the ACT tick, so their single wait slot
            # takes the DMA directly and the funnel is unnecessary.
            if mul_eng not in prev_mul:
                jd = fpool.tile([1, 1], FP32)
                fun = mul_eng.tensor_copy(jd, h_h[0:1, 0, 0:1])
                pin = fun
            else:
                fun = None
                pin = prev_mul[mul_eng]
            prod = prpool.tile([P, jh, D], FP32, tag="prod")
            wsrc = bass.AP(
                tensor=w_base.tensor, offset=w_base.offset,
                ap=[w_base.ap[0], [0, jh], w_base.ap[1]],
            )
            mul = mul_eng.tensor_mul(prod, h_h, wsrc)
            add_dep_helper(mul.ins, pin.ins, sync=False, reason="mult stream order")
            prev_mul[mul_eng] = mul
            nc.vector.tensor_reduce(
                out=t_sup[:, hh * jh : (hh + 1) * jh],
                in_=prod,
                axis=mybir.AxisListType.X,
                op=mybir.AluOpType.add,
            )
        p_sup = tppool.tile([P, SUP], FP32)
        nc.scalar.activation(
            out=p_sup,
            in_=t_sup,
            func=mybir.ActivationFunctionType.Exp,
            bias=bias_sh,
            scale=1.0,
        )
        # PE prefetch touching each h tile: absorbs the DMA wait so the
        # first real u-matmul on that tile carries only the ACT wait.
        pres = [
            nc.tensor.matmul(
                junk_ps, lhsT=hb[:, 0, 0:1], rhs=hb[:, 0, 0:1], start=True, stop=True
            )
            for hb in halves
        ]
        for j in range(SUP):
            k = s * SUP + j
            hb = halves[j // jh]
            mm = nc.tensor.matmul(
                u_ps,
                lhsT=hb[:, j % jh, :],
                rhs=p_sup[:, j : j + 1],
                start=(k == 0),
                stop=(k == NT - 1),
                skip_group_check=True,
            )
            if j % jh == 0:
                add_dep_helper(
                    mm.ins, pres[j // jh].ins, sync=False, reason="pe sees h dma"
                )
        for j in range(SUP):
            nc.tensor.matmul(
                z_ps,
                lhsT=ones_zbc,
                rhs=p_sup[:, j : j + 1],
                start=(s == 0 and j == 0),
                stop=(s == NSUP - 1 and j == SUP - 1),
                skip_group_check=True,
            )

    # --- finale: 1/Z on u's partitions, ybc = (u/Z broadcast) @ W.T,
    # one [128, 64] scaled row copy, out-DMA replicates it 8x per partition
    # via a stride-0 source read (descriptor-gen starts ~1.5 us earlier than
    # materializing [128, 8, 64] first).
    rec64 = spool.tile([D, 1], FP32)
    nc.vector.reciprocal(rec64, z_ps)
    u_sb = spool.tile([D, 1], FP32)
    nc.scalar.copy(u_sb, u_ps)
    u_sc = spool.tile([D, 1], FP32)
    nc.vector.tensor_mul(u_sc, u_sb, rec64)
    ubc = bass.AP(tensor=u_sc.tensor, offset=u_sc.offset,
                  ap=[u_sc.ap[0], [0, P]])
    ybc_ps = ppool.tile([P, D], FP32)
    nc.tensor.matmul(ybc_ps, lhsT=ubc, rhs=WT_sb, start=True, stop=True)
    y_sb = spool.tile([P, D], FP32)
    nc.vector.tensor_copy(y_sb, ybc_ps)
    ysrc = bass.AP(tensor=y_sb.tensor, offset=y_sb.offset,
                   ap=[y_sb.ap[0], [0, OUT_J], y_sb.ap[1]])
    nc.sync.dma_start(out=out.rearrange("(p j) d -> p j d", j=OUT_J), in_=ysrc)


def build_bass():
    nc = bacc.Bacc("TRN2", debug=False, target_bir_lowering=False)
    h = nc.dram_tensor("h", [N, D], FP32, kind="ExternalInput").ap()
    Wa = nc.dram_tensor("Wa", [D, 2 * D + 1], FP32, kind="ExternalInput").ap()
    out = nc.dram_tensor("out", [ROWS_PER_CORE, D], FP32, kind="ExternalOutput").ap()
    with tile.TileContext(nc) as tc:
        with ExitStack() as ctx:
            build_kernel(ctx, tc, h, Wa, out)
    nc.compile()
    return nc


_NC_CACHE = None


def _get_nc():
    global _NC_CACHE
    if _NC_CACHE is None:
        _NC_CACHE = build_bass()
    return _NC_CACHE


def kernel(**inputs) -> np.ndarray:
    h = np.ascontiguousarray(np.asarray(inputs["h"], dtype=np.float32))
    W = np.ascontiguousarray(np.asarray(inputs["W"], dtype=np.float32))
    aw = np.ascontiguousarray(np.asarray(inputs["attn_w"], dtype=np.float32))
    assert h.shape == (N, D) and W.shape == (D, D) and aw.shape == (1, 2 * D)

    Wa = np.ascontiguousarray(
        np.concatenate([W, aw[0, D : 2 * D][:, None], W.T], axis=1)
    )
    nc = _get_nc()
    in_map = {"h": h, "Wa": Wa}
    in_maps = [in_map for _ in range(NCORES)]
    res = bass_utils.run_bass_kernel_spmd(nc, in_maps, list(range(NCORES)))
    blocks = [res.results[i]["out"] for i in range(NCORES)]
    return np.concatenate(blocks, axis=0)


if __name__ == "__main__":
    nc = _get_nc()
    print("Bass program built OK")



# revision 10
# speedup vs baseline: 1.0812x; 1.0508x over previous
"""Trainium2 Bass kernel for nn_GATLayer (gnn_message_passing).

v2 deltas over the original 18794 ns baseline (sim + HW verified 18445 ns):
  * Wa = [W | a_r^col | W.T] is ONE host-concatenated const tile (pure
    layout, no host math), so the w_t chain needs no separate a_r DMA and
    w_base is built by a single matmul: stationary a_col broadcast across
    128 columns via a stride-0 free dim against W, plus one PSUM->SBUF copy
    (replaces the 4-hop wt/wrep chain and the DVE W-transposes; W.T arrives
    in the same tile).
  * The first h super-tile streams as two half-DMAs so the mult chain
    starts ~360 ns earlier against the now-early w_base.
  * Finale: 1/Z broadcast to all partitions with one reciprocal + one
    stride-0 stationary matmul; ybc = (u broadcast) @ W.T in one matmul
    (stationary u with a stride-0 free dim); the [128, 8, 64] out tile is
    materialized by DVE/ACT copies that apply the 1/Z scale.

Math: the reference computes
    Wh = h @ W.T
    e[i, j] = (Wh @ a_l)[i] + (Wh @ a_r)[j] + b
    out = softmax(e, axis=1) @ Wh
Because e[i, :] = const_i + t where t = Wh @ a_r, every softmax row equals
softmax(t) exactly (row-constant shifts cancel).  Hence
    out[i, :] = softmax(t) @ Wh = ((p @ h) @ W.T) / sum(p),  p = exp(t - c)
for any constant c, identical for all rows i.  The kernel computes the
weighted column-sum u = p @ h and Z = sum(p) in one streaming pass over h,
then projects y = (u @ W.T) / Z and broadcasts y to its row block.

Layout: h streams in 8 super-tiles of [128, 8, 64] where partition p holds 8
CONSECUTIVE rows (row = 1024*s + 8*p + j) — both DMA sides are then fully
linear (no sub-512B segments).  The u-matmul contracts row-class j across
partitions; PSUM accumulation over all (s, j) gives the same u.

Engine split: t = h@w_t runs as elementwise mult (DVE for the 5 earliest-
arriving super-tiles, Pool for the last 3) + segmented reduce (DVE);  exp on
ACT (with accum_out Z partials, except the tail tile);  u-accumulation on PE
(h_s as stationary, p column as moving);  finale on PE/DVE/ACT.

Toolchain constraint: walrus allows ONE sync-wait per instruction
(bacc legalizes the rest into EventSemaphores).  The tiny "funnel" copies
and junk prefetch matmuls absorb DMA-queue waits so every hot instruction
carries at most one; DMA queues are routed so no queue is reused.

Each of the 8 cores runs the same program and emits one [1024, 64] row block
(all blocks are equal); the host concatenates them to the full [8192, 64].
"""

import sys
from contextlib import ExitStack

sys.path.insert(0, "/opt/trn_rl_repo")

import numpy as np

import concourse.bass as bass
import concourse.mybir as mybir
import concourse.tile as tile
from concourse import bacc, bass_utils
from concourse.tile_rust import add_dep_helper

N = 8192
D = 64
NCORES = 8
ROWS_PER_CORE = N // NCORES  # 1024
P = 128                      # SBUF partitions
NT = N // P                  # 64 row tiles of h
SUP = 8                      # row tiles per super-tile
NSUP = NT // SUP             # 8 super-tiles
# t-mult engine per super-tile: DVE takes the first four (earliest DMA
# arrivals; Pool cannot start before ~h4 lands anyway and its mults cost ~2x,
# so a bigger Pool share only builds backlog behind the last arrival).
DVE_MULT_TILES = {0, 1, 2, 3, 4}
FP32 = mybir.dt.float32
EXP_SHIFT = -8.0             # exp(t - 8): cancels in u/Z, guards overflow


OUT_J = ROWS_PER_CORE // P   # 8 output rows per partition


def _bcast_out(t):
    """View a [P, D] tile as [P, OUT_J, D] via a step-0 middle dim."""
    return bass.AP(tensor=t.tensor, offset=t.offset, ap=[t.ap[0], [0, OUT_J], t.ap[1]])


def build_kernel(ctx, tc, h, Wa, out):
    nc = tc.nc
    const = ctx.enter_context(tc.tile_pool(name="const", bufs=1))
    hpool = ctx.enter_context(tc.tile_pool(name="hbuf", bufs=NSUP))
    prpool = ctx.enter_context(tc.tile_pool(name="prod", bufs=NSUP))
    spool = ctx.enter_context(tc.tile_pool(name="scratch", bufs=2))
    tppool = ctx.enter_context(tc.tile_pool(name="tp", bufs=NSUP))
    fpool = ctx.enter_context(tc.tile_pool(name="funnel", bufs=NSUP))
    ppool = ctx.enter_context(tc.tile_pool(name="psum", bufs=1, space="PSUM"))
    upool = ctx.enter_context(tc.tile_pool(name="upsum", bufs=1, space="PSUM"))

    # --- constants.  One host-concatenated tile Wa = [W | a_r^col | W.T]
    # (layout only) via SWDGE; the whole w chain is ONE matmul: stationary
    # a_col broadcast across 128 columns (stride-0 free dim) against W gives
    # w_t replicated on every partition, plus one PSUM->SBUF copy.
    Wa_sb = const.tile([D, 2 * D + 2], FP32)
    nc.gpsimd.dma_start(out=Wa_sb[:, 0 : 2 * D + 1], in_=Wa)
    Wsb = Wa_sb[:, 0:D]
    a_col = Wa_sb[:, D : D + 1]
    WT_sb = Wa_sb[:, D + 1 : 2 * D + 1]

    # scatter metadata + descriptor prep (gen early; trigger fires at the end)
    out_sb = spool.tile([P, OUT_J, D], FP32)

    junk_ps = upool.tile([1, 1], FP32)
    ones_col = const.tile([P, 1], FP32)
    nc.vector.memset(ones_col, 1.0)
    bias_sh = const.tile([P, 1], FP32)
    nc.vector.memset(bias_sh, EXP_SHIFT)

    acol_bc = bass.AP(tensor=Wa_sb.tensor, offset=a_col.offset,
                      ap=[a_col.ap[0], [0, P]])
    wb_ps = ppool.tile([P, D], FP32)
    pre_w = nc.tensor.matmul(
        junk_ps, lhsT=Wa_sb[:, 0:1], rhs=Wa_sb[:, 0:1], start=True, stop=True
    )
    wb_mm = nc.tensor.matmul(wb_ps, lhsT=acol_bc, rhs=Wsb, start=True, stop=True)
    add_dep_helper(wb_mm.ins, pre_w.ins, sync=False, reason="pe sees Wa dma first")
    w_base = const.tile([P, D], FP32)
    nc.vector.tensor_copy(w_base, wb_ps)

    u_ps = upool.tile([D, 1], FP32)
    z_ps = upool.tile([D, 1], FP32)
    ones_zbc = bass.AP(tensor=ones_col.tensor, offset=ones_col.offset,
                       ap=[ones_col.ap[0], [0, D]])

    # h viewed as [s][p][j][d], row = 1024*s + 8*p + j: fully linear DMA
    hview = h.rearrange("(s p j) d -> s p j d", j=SUP, p=P)

    prev_mul = {}
    for s in range(NSUP):
        # s=0 streams in two half-DMAs so the first mult starts ~a half-
        # transfer earlier; the whole DVE chain (which sets the steady-state
        # floor) slides left with it.
        nhalf = 2 if s in (0, NSUP - 1) else 1  # split first (early start) and last (short tail)
        jh = SUP // nhalf
        halves = []
        for hh in range(nhalf):
            h_h = hpool.tile([P, jh, D], FP32, tag="hbuf")
            nc.sync.dma_start(out=h_h, in_=hview[s][:, hh * jh : (hh + 1) * jh, :])
            halves.append(h_h)
        # t-path: prod = h * w_t (broadcast), then segmented sum over d.
        # Order-only dep edges chain each engine's mults in arrival order.
        mul_eng = nc.vector if s in DVE_MULT_TILES else nc.gpsimd
        t_sup = tppool.tile([P, SUP], FP32)
        for hh, h_h in enumerate(halves):
            # Only each engine's FIRST mult needs a funnel: it waits on ACT
            # (w_base), with the funnel absorbing the DMA wait.  Later mults
            # have already observed the ACT tick, so their single wait slot
            # takes the DMA directly and the funnel is unnecessary.
            if mul_eng not in prev_mul:
                jd = fpool.tile([1, 1], FP32)
                fun = mul_eng.tensor_copy(jd, h_h[0:1, 0, 0:1])
                pin = fun
            else:
                fun = None
                pin = prev_mul[mul_eng]
            prod = prpool.tile([P, jh, D], FP32, tag="prod")
            wsrc = bass.AP(
                tensor=w_base.tensor, offset=w_base.offset,
                ap=[w_base.ap[0], [0, jh], w_base.ap[1]],
            )
            mul = mul_eng.tensor_mul(prod, h_h, wsrc)
            add_dep_helper(mul.ins, pin.ins, sync=False, reason="mult stream order")
            prev_mul[mul_eng] = mul
            nc.vector.tensor_reduce(
                out=t_sup[:, hh * jh : (hh + 1) * jh],
                in_=prod,
                axis=mybir.AxisListType.X,
                op=mybir.AluOpType.add,
            )
        p_sup = tppool.tile([P, SUP], FP32)
        nc.scalar.activation(
            out=p_sup,
            in_=t_sup,
            func=mybir.ActivationFunctionType.Exp,
            bias=bias_sh,
            scale=1.0,
        )
        # PE prefetch touching each h tile: absorbs the DMA wait so the
        # first real u-matmul on that tile carries only the ACT wait.
        pres = [
            nc.tensor.matmul(
                junk_ps, lhsT=hb[:, 0, 0:1], rhs=hb[:, 0, 0:1], start=True, stop=True
            )
            for hb in halves
        ]
        for j in range(SUP):
            k = s * SUP + j
            hb = halves[j // jh]
            mm = nc.tensor.matmul(
                u_ps,
                lhsT=hb[:, j % jh, :],
                rhs=p_sup[:, j : j + 1],
                start=(k == 0),
                stop=(k == NT - 1),
                skip_group_check=True,
            )
            if j % jh == 0:
                add_dep_helper(
                    mm.ins, pres[j // jh].ins, sync=False, reason="pe sees h dma"
                )
        for j in range(SUP):
            nc.tensor.matmul(
                z_ps,
                lhsT=ones_zbc,
                rhs=p_sup[:, j : j + 1],
                start=(s == 0 and j == 0),
                stop=(s == NSUP - 1 and j == SUP - 1),
                skip_group_check=True,
            )

    # --- finale: 1/Z on u's partitions, ybc = (u/Z broadcast) @ W.T,
    # one [128, 64] scaled row copy, out-DMA replicates it 8x per partition
    # via a stride-0 source read (descriptor-gen starts ~1.5 us earlier than
    # materializing [128, 8, 64] first).
    rec64 = spool.tile([D, 1], FP32)
    nc.vector.reciprocal(rec64, z_ps)
    u_sb = spool.tile([D, 1], FP32)
    nc.vector.tensor_copy(u_sb, u_ps)
    u_sc = spool.tile([D, 1], FP32)
    nc.vector.tensor_mul(u_sc, u_sb, rec64)
    ubc = bass.AP(tensor=u_sc.tensor, offset=u_sc.offset,
                  ap=[u_sc.ap[0], [0, P]])
    ybc_ps = ppool.tile([P, D], FP32)
    nc.tensor.matmul(ybc_ps, lhsT=ubc, rhs=WT_sb, start=True, stop=True)
    y2_sb = spool.tile([P, 2, D], FP32)
    ybc2 = bass.AP(tensor=ybc_ps.tensor, offset=ybc_ps.offset,
                   ap=[ybc_ps.ap[0], [0, 2], ybc_ps.ap[1]])
    nc.vector.tensor_copy(y2_sb, ybc2)
    ysrc = bass.AP(tensor=y2_sb.tensor, offset=y2_sb.offset,
                   ap=[y2_sb.ap[0], [0, OUT_J // 2], [1, 2 * D]])
    nc.sync.dma_start(
        out=out.rearrange("(p j) d -> p j d", j=OUT_J // 2), in_=ysrc
    )


def build_bass():
    nc = bacc.Bacc("TRN2", debug=False, target_bir_lowering=False)
    h = nc.dram_tensor("h", [N, D], FP32, kind="ExternalInput").ap()
    Wa = nc.dram_tensor("Wa", [D, 2 * D + 1], FP32, kind="ExternalInput").ap()
    out = nc.dram_tensor("out", [ROWS_PER_CORE, D], FP32, kind="ExternalOutput").ap()
    with tile.TileContext(nc) as tc:
        with ExitStack() as ctx:
            build_kernel(ctx, tc, h, Wa, out)
    nc.compile()
    return nc


_NC_CACHE = None


def _get_nc():
    global _NC_CACHE
    if _NC_CACHE is None:
        _NC_CACHE = build_bass()
    return _NC_CACHE


def kernel(**inputs) -> np.ndarray:
    h = np.ascontiguousarray(np.asarray(inputs["h"], dtype=np.float32))
    W = np.ascontiguousarray(np.asarray(inputs["W"], dtype=np.float32))
    aw = np.ascontiguousarray(np.asarray(inputs["attn_w"], dtype=np.float32))
    assert h.shape == (N, D) and W.shape == (D, D) and aw.shape == (1, 2 * D)

    Wa = np.ascontiguousarray(
        np.concatenate([W, aw[0, D : 2 * D][:, None], W.T], axis=1)
    )
    nc = _get_nc()
    in_map = {"h": h, "Wa": Wa}
    in_maps = [in_map for _ in range(NCORES)]
    res = bass_utils.run_bass_kernel_spmd(nc, in_maps, list(range(NCORES)))
    blocks = [res.results[i]["out"] for i in range(NCORES)]
    return np.concatenate(blocks, axis=0)


if __name__ == "__main__":
    nc = _get_nc()
    print("Bass program built OK")



# revision 11
# speedup vs baseline: 1.0911x; 1.0092x over previous
"""Trainium2 Bass kernel for nn_GATLayer (gnn_message_passing).

v2 deltas over the original 18794 ns baseline (sim + HW verified 18445 ns):
  * Wa = [W | a_r^col | W.T] is ONE host-concatenated const tile (pure
    layout, no host math), so the w_t chain needs no separate a_r DMA and
    w_base is built by a single matmul: stationary a_col broadcast across
    128 columns via a stride-0 free dim against W, plus one PSUM->SBUF copy
    (replaces the 4-hop wt/wrep chain and the DVE W-transposes; W.T arrives
    in the same tile).
  * The first h super-tile streams as two half-DMAs so the mult chain
    starts ~360 ns earlier against the now-early w_base.
  * Finale: 1/Z broadcast to all partitions with one reciprocal + one
    stride-0 stationary matmul; ybc = (u broadcast) @ W.T in one matmul
    (stationary u with a stride-0 free dim); the [128, 8, 64] out tile is
    materialized by DVE/ACT copies that apply the 1/Z scale.

Math: the reference computes
    Wh = h @ W.T
    e[i, j] = (Wh @ a_l)[i] + (Wh @ a_r)[j] + b
    out = softmax(e, axis=1) @ Wh
Because e[i, :] = const_i + t where t = Wh @ a_r, every softmax row equals
softmax(t) exactly (row-constant shifts cancel).  Hence
    out[i, :] = softmax(t) @ Wh = ((p @ h) @ W.T) / sum(p),  p = exp(t - c)
for any constant c, identical for all rows i.  The kernel computes the
weighted column-sum u = p @ h and Z = sum(p) in one streaming pass over h,
then projects y = (u @ W.T) / Z and broadcasts y to its row block.

Layout: h streams in 8 super-tiles of [128, 8, 64] where partition p holds 8
CONSECUTIVE rows (row = 1024*s + 8*p + j) — both DMA sides are then fully
linear (no sub-512B segments).  The u-matmul contracts row-class j across
partitions; PSUM accumulation over all (s, j) gives the same u.

Engine split: t = h@w_t runs as elementwise mult (DVE for the 5 earliest-
arriving super-tiles, Pool for the last 3) + segmented reduce (DVE);  exp on
ACT (with accum_out Z partials, except the tail tile);  u-accumulation on PE
(h_s as stationary, p column as moving);  finale on PE/DVE/ACT.

Toolchain constraint: walrus allows ONE sync-wait per instruction
(bacc legalizes the rest into EventSemaphores).  The tiny "funnel" copies
and junk prefetch matmuls absorb DMA-queue waits so every hot instruction
carries at most one; DMA queues are routed so no queue is reused.

Each of the 8 cores runs the same program and emits one [1024, 64] row block
(all blocks are equal); the host concatenates them to the full [8192, 64].
"""

import sys
from contextlib import ExitStack

sys.path.insert(0, "/opt/trn_rl_repo")

import numpy as np

import concourse.bass as bass
import concourse.mybir as mybir
import concourse.tile as tile
from concourse import bacc, bass_utils
from concourse.tile_rust import add_dep_helper

N = 8192
D = 64
NCORES = 8
ROWS_PER_CORE = N // NCORES  # 1024
P = 128                      # SBUF partitions
NT = N // P                  # 64 row tiles of h
SUP = 8                      # row tiles per super-tile
NSUP = NT // SUP             # 8 super-tiles
# t-mult engine per super-tile: DVE takes the first four (earliest DMA
# arrivals; Pool cannot start before ~h4 lands anyway and its mults cost ~2x,
# so a bigger Pool share only builds backlog behind the last arrival).
DVE_MULT_TILES = {0, 1, 2, 3, 4}
HYBRID_TILES = {2, 4}        # d-reduce via PE transpose + ACT copy, not DVE        # d-reduce via PE transpose + ACT copy, not DVE
FP32 = mybir.dt.float32
EXP_SHIFT = -8.0             # exp(t - 8): cancels in u/Z, guards overflow


OUT_J = ROWS_PER_CORE // P   # 8 output rows per partition


def _bcast_out(t):
    """View a [P, D] tile as [P, OUT_J, D] via a step-0 middle dim."""
    return bass.AP(tensor=t.tensor, offset=t.offset, ap=[t.ap[0], [0, OUT_J], t.ap[1]])


def build_kernel(ctx, tc, h, Wa, out):
    nc = tc.nc
    const = ctx.enter_context(tc.tile_pool(name="const", bufs=1))
    hpool = ctx.enter_context(tc.tile_pool(name="hbuf", bufs=NSUP))
    prpool = ctx.enter_context(tc.tile_pool(name="prod", bufs=NSUP))
    spool = ctx.enter_context(tc.tile_pool(name="scratch", bufs=2))
    tppool = ctx.enter_context(tc.tile_pool(name="tp", bufs=NSUP))
    fpool = ctx.enter_context(tc.tile_pool(name="funnel", bufs=NSUP))
    ppool = ctx.enter_context(tc.tile_pool(name="psum", bufs=1, space="PSUM"))
    upool = ctx.enter_context(tc.tile_pool(name="upsum", bufs=1, space="PSUM"))
    tpsum = ctx.enter_context(tc.tile_pool(name="tpsum", bufs=1, space="PSUM"))

    # --- constants.  One host-concatenated tile Wa = [W | a_r^col | W.T]
    # (layout only) via SWDGE; the whole w chain is ONE matmul: stationary
    # a_col broadcast across 128 columns (stride-0 free dim) against W gives
    # w_t replicated on every partition, plus one PSUM->SBUF copy.
    Wa_sb = const.tile([D, 2 * D + 2], FP32)
    nc.gpsimd.dma_start(out=Wa_sb[:, 0 : 2 * D + 1], in_=Wa)
    Wsb = Wa_sb[:, 0:D]
    a_col = Wa_sb[:, D : D + 1]
    WT_sb = Wa_sb[:, D + 1 : 2 * D + 1]

    # scatter metadata + descriptor prep (gen early; trigger fires at the end)
    out_sb = spool.tile([P, OUT_J, D], FP32)

    junk_ps = upool.tile([1, 1], FP32)
    ones_col = const.tile([P, 1], FP32)
    nc.vector.memset(ones_col, 1.0)
    bias_sh = const.tile([P, 1], FP32)
    nc.vector.memset(bias_sh, EXP_SHIFT)

    acol_bc = bass.AP(tensor=Wa_sb.tensor, offset=a_col.offset,
                      ap=[a_col.ap[0], [0, P]])
    wb_ps = ppool.tile([P, D], FP32)
    pre_w = nc.tensor.matmul(
        junk_ps, lhsT=Wa_sb[:, 0:1], rhs=Wa_sb[:, 0:1], start=True, stop=True
    )
    wb_mm = nc.tensor.matmul(wb_ps, lhsT=acol_bc, rhs=Wsb, start=True, stop=True)
    add_dep_helper(wb_mm.ins, pre_w.ins, sync=False, reason="pe sees Wa dma first")
    w_base = const.tile([P, D], FP32)
    nc.vector.tensor_copy(w_base, wb_ps)

    iota2d = const.tile([P, P], mybir.dt.int32)
    nc.gpsimd.iota(iota2d, pattern=[[1, P]], base=0, channel_multiplier=-1)
    ident = const.tile([P, P], mybir.dt.bfloat16)
    nc.vector.tensor_scalar(out=ident, in0=iota2d, scalar1=0, scalar2=None,
                            op0=mybir.AluOpType.is_equal)
    onesb_col = const.tile([P, 1], mybir.dt.bfloat16)
    nc.vector.memset(onesb_col, 1.0)
    u_ps = upool.tile([D, 1], FP32)
    z_ps = upool.tile([D, 1], FP32)
    ones_zbc = bass.AP(tensor=ones_col.tensor, offset=ones_col.offset,
                       ap=[ones_col.ap[0], [0, D]])

    # h viewed as [s][p][j][d], row = 1024*s + 8*p + j: fully linear DMA
    hview = h.rearrange("(s p j) d -> s p j d", j=SUP, p=P)

    prev_mul = {}
    for s in range(NSUP):
        # s=0 streams in two half-DMAs so the first mult starts ~a half-
        # transfer earlier; the whole DVE chain (which sets the steady-state
        # floor) slides left with it.
        nhalf = 2 if s in (0, NSUP - 1) else 1  # split first (early start) and last (short tail)
        jh = SUP // nhalf
        halves = []
        for hh in range(nhalf):
            h_h = hpool.tile([P, jh, D], FP32, tag="hbuf")
            nc.sync.dma_start(out=h_h, in_=hview[s][:, hh * jh : (hh + 1) * jh, :])
            halves.append(h_h)
        # t-path: prod = h * w_t (broadcast), then segmented sum over d.
        # Order-only dep edges chain each engine's mults in arrival order.
        mul_eng = nc.vector if s in DVE_MULT_TILES else nc.gpsimd
        t_sup = tppool.tile([P, SUP], FP32)
        t_srcs = []
        for hh, h_h in enumerate(halves):
            # Only each engine's FIRST mult needs a funnel: it waits on ACT
            # (w_base), with the funnel absorbing the DMA wait.  Later mults
            # have already observed the ACT tick, so their single wait slot
            # takes the DMA directly and the funnel is unnecessary.
            if mul_eng not in prev_mul:
                jd = fpool.tile([1, 1], FP32)
                fun = mul_eng.tensor_copy(jd, h_h[0:1, 0, 0:1])
                pin = fun
            else:
                fun = None
                pin = prev_mul[mul_eng]
            hybrid = s in HYBRID_TILES
            pdt = mybir.dt.bfloat16 if hybrid else FP32
            prod = prpool.tile([P, jh * D], pdt, tag="prod")
            prod3 = bass.AP(tensor=prod.tensor, offset=prod.offset,
                            ap=[prod.ap[0], [D, jh], [1, D]])
            wsrc = bass.AP(
                tensor=w_base.tensor, offset=w_base.offset,
                ap=[w_base.ap[0], [0, jh], w_base.ap[1]],
            )
            mul = mul_eng.tensor_mul(prod3, h_h, wsrc)
            add_dep_helper(mul.ins, pin.ins, sync=False, reason="mult stream order")
            prev_mul[mul_eng] = mul
            if hybrid:
                # PE transposes prod (bf16, 1 cyc/row) into PSUM; ACT copies
                # it back; per-j ones-matmuls reduce d (now on partitions)
                nfree = jh * D
                nblk = nfree // P
                pT_ps = tpsum.tile([P, nfree], mybir.dt.bfloat16, tag="pT")
                for b in range(nblk):
                    nc.tensor.transpose(
                        pT_ps[:, b * P : (b + 1) * P],
                        prod[:, b * P : (b + 1) * P],
                        ident,
                    )
                pT_sb = prpool.tile([P, nfree], mybir.dt.bfloat16, tag="pTsb")
                nc.scalar.copy(pT_sb, pT_ps)
                t_hyb = tpsum.tile([P, jh], FP32, tag="thyb")
                for j in range(jh):
                    fo = j * D
                    b, po = fo // P, fo % P
                    nc.tensor.matmul(
                        t_hyb[:, j : j + 1],
                        lhsT=pT_sb[po : po + D, b * P : (b + 1) * P],
                        rhs=onesb_col[po : po + D],
                        start=True, stop=True, skip_group_check=True,
                    )
                t_srcs.append((hh * jh, t_hyb))
            else:
                nc.vector.tensor_reduce(
                    out=t_sup[:, hh * jh : (hh + 1) * jh],
                    in_=prod3,
                    axis=mybir.AxisListType.X,
                    op=mybir.AluOpType.add,
                )
        p_sup = tppool.tile([P, SUP], FP32)
        if not t_srcs:
            nc.scalar.activation(
                out=p_sup, in_=t_sup,
                func=mybir.ActivationFunctionType.Exp,
                bias=bias_sh, scale=1.0,
            )
        else:
            for (joff, th) in t_srcs:
                jn = th.shape[1]
                nc.scalar.activation(
                    out=p_sup[:, joff : joff + jn], in_=th,
                    func=mybir.ActivationFunctionType.Exp,
                    bias=bias_sh, scale=1.0,
                )
        # PE prefetch touching each h tile: absorbs the DMA wait so the
        # first real u-matmul on that tile carries only the ACT wait.
        pres = [
            nc.tensor.matmul(
                junk_ps, lhsT=hb[:, 0, 0:1], rhs=hb[:, 0, 0:1], start=True, stop=True
            )
            for hb in halves
        ]
        for j in range(SUP):
            k = s * SUP + j
            hb = halves[j // jh]
            mm = nc.tensor.matmul(
                u_ps,
                lhsT=hb[:, j % jh, :],
                rhs=p_sup[:, j : j + 1],
                start=(k == 0),
                stop=(k == NT - 1),
                skip_group_check=True,
            )
            if j % jh == 0:
                add_dep_helper(
                    mm.ins, pres[j // jh].ins, sync=False, reason="pe sees h dma"
                )
        for j in range(SUP):
            nc.tensor.matmul(
                z_ps,
                lhsT=ones_zbc,
                rhs=p_sup[:, j : j + 1],
                start=(s == 0 and j == 0),
                stop=(s == NSUP - 1 and j == SUP - 1),
                skip_group_check=True,
            )

    # --- finale: 1/Z on u's partitions, ybc = (u/Z broadcast) @ W.T,
    # one [128, 64] scaled row copy, out-DMA replicates it 8x per partition
    # via a stride-0 source read (descriptor-gen starts ~1.5 us earlier than
    # materializing [128, 8, 64] first).
    rec64 = spool.tile([D, 1], FP32)
    nc.vector.reciprocal(rec64, z_ps)
    u_sb = spool.tile([D, 1], FP32)
    nc.vector.tensor_copy(u_sb, u_ps)
    u_sc = spool.tile([D, 1], FP32)
    nc.vector.tensor_mul(u_sc, u_sb, rec64)
    ubc = bass.AP(tensor=u_sc.tensor, offset=u_sc.offset,
                  ap=[u_sc.ap[0], [0, P]])
    ybc_ps = ppool.tile([P, D], FP32)
    nc.tensor.matmul(ybc_ps, lhsT=ubc, rhs=WT_sb, start=True, stop=True)
    y2_sb = spool.tile([P, 2, D], FP32)
    ybc2 = bass.AP(tensor=ybc_ps.tensor, offset=ybc_ps.offset,
                   ap=[ybc_ps.ap[0], [0, 2], ybc_ps.ap[1]])
    nc.vector.tensor_copy(y2_sb, ybc2)
    ysrc = bass.AP(tensor=y2_sb.tensor, offset=y2_sb.offset,
                   ap=[y2_sb.ap[0], [0, OUT_J // 2], [1, 2 * D]])
    nc.sync.dma_start(
        out=out.rearrange("(p j) d -> p j d", j=OUT_J // 2), in_=ysrc
    )


def build_bass():
    nc = bacc.Bacc("TRN2", debug=False, target_bir_lowering=False)
    h = nc.dram_tensor("h", [N, D], FP32, kind="ExternalInput").ap()
    Wa = nc.dram_tensor("Wa", [D, 2 * D + 1], FP32, kind="ExternalInput").ap()
    out = nc.dram_tensor("out", [ROWS_PER_CORE, D], FP32, kind="ExternalOutput").ap()
    with tile.TileContext(nc) as tc:
        with ExitStack() as ctx:
            build_kernel(ctx, tc, h, Wa, out)
    nc.compile()
    return nc


_NC_CACHE = None


def _get_nc():
    global _NC_CACHE
    if _NC_CACHE is None:
        _NC_CACHE = build_bass()
    return _NC_CACHE


def kernel(**inputs) -> np.ndarray:
    h = np.ascontiguousarray(np.asarray(inputs["h"], dtype=np.float32))
    W = np.ascontiguousarray(np.asarray(inputs["W"], dtype=np.float32))
    aw = np.ascontiguousarray(np.asarray(inputs["attn_w"], dtype=np.float32))
    assert h.shape == (N, D) and W.shape == (D, D) and aw.shape == (1, 2 * D)

    Wa = np.ascontiguousarray(
        np.concatenate([W, aw[0, D : 2 * D][:, None], W.T], axis=1)
    )
    nc = _get_nc()
    in_map = {"h": h, "Wa": Wa}
    in_maps = [in_map for _ in range(NCORES)]
    res = bass_utils.run_bass_kernel_spmd(nc, in_maps, list(range(NCORES)))
    blocks = [res.results[i]["out"] for i in range(NCORES)]
    return np.concatenate(blocks, axis=0)


if __name__ == "__main__":
    nc = _get_nc()
    print("Bass program built OK")



# revision 12
# speedup vs baseline: 1.0995x; 1.0077x over previous
"""Trainium2 Bass kernel for nn_GATLayer (gnn_message_passing).

v2 deltas over the original 18794 ns baseline (sim + HW verified 18445 ns):
  * Wa = [W | a_r^col | W.T] is ONE host-concatenated const tile (pure
    layout, no host math), so the w_t chain needs no separate a_r DMA and
    w_base is built by a single matmul: stationary a_col broadcast across
    128 columns via a stride-0 free dim against W, plus one PSUM->SBUF copy
    (replaces the 4-hop wt/wrep chain and the DVE W-transposes; W.T arrives
    in the same tile).
  * The first h super-tile streams as two half-DMAs so the mult chain
    starts ~360 ns earlier against the now-early w_base.
  * Finale: 1/Z broadcast to all partitions with one reciprocal + one
    stride-0 stationary matmul; ybc = (u broadcast) @ W.T in one matmul
    (stationary u with a stride-0 free dim); the [128, 8, 64] out tile is
    materialized by DVE/ACT copies that apply the 1/Z scale.

Math: the reference computes
    Wh = h @ W.T
    e[i, j] = (Wh @ a_l)[i] + (Wh @ a_r)[j] + b
    out = softmax(e, axis=1) @ Wh
Because e[i, :] = const_i + t where t = Wh @ a_r, every softmax row equals
softmax(t) exactly (row-constant shifts cancel).  Hence
    out[i, :] = softmax(t) @ Wh = ((p @ h) @ W.T) / sum(p),  p = exp(t - c)
for any constant c, identical for all rows i.  The kernel computes the
weighted column-sum u = p @ h and Z = sum(p) in one streaming pass over h,
then projects y = (u @ W.T) / Z and broadcasts y to its row block.

Layout: h streams in 8 super-tiles of [128, 8, 64] where partition p holds 8
CONSECUTIVE rows (row = 1024*s + 8*p + j) — both DMA sides are then fully
linear (no sub-512B segments).  The u-matmul contracts row-class j across
partitions; PSUM accumulation over all (s, j) gives the same u.

Engine split: t = h@w_t runs as elementwise mult (DVE for the 5 earliest-
arriving super-tiles, Pool for the last 3) + segmented reduce (DVE);  exp on
ACT (with accum_out Z partials, except the tail tile);  u-accumulation on PE
(h_s as stationary, p column as moving);  finale on PE/DVE/ACT.

Toolchain constraint: walrus allows ONE sync-wait per instruction
(bacc legalizes the rest into EventSemaphores).  The tiny "funnel" copies
and junk prefetch matmuls absorb DMA-queue waits so every hot instruction
carries at most one; DMA queues are routed so no queue is reused.

Each of the 8 cores runs the same program and emits one [1024, 64] row block
(all blocks are equal); the host concatenates them to the full [8192, 64].
"""

import sys
from contextlib import ExitStack

sys.path.insert(0, "/opt/trn_rl_repo")

import numpy as np

import concourse.bass as bass
import concourse.mybir as mybir
import concourse.tile as tile
from concourse import bacc, bass_utils
from concourse.tile_rust import add_dep_helper

N = 8192
D = 64
NCORES = 8
ROWS_PER_CORE = N // NCORES  # 1024
P = 128                      # SBUF partitions
NT = N // P                  # 64 row tiles of h
SUP = 8                      # row tiles per super-tile
NSUP = NT // SUP             # 8 super-tiles
# t-mult engine per super-tile: DVE takes the first four (earliest DMA
# arrivals; Pool cannot start before ~h4 lands anyway and its mults cost ~2x,
# so a bigger Pool share only builds backlog behind the last arrival).
DVE_MULT_TILES = {0, 1, 2, 3, 4, 5}
HYBRID_TILES = {2, 4}        # d-reduce via PE transpose + ACT copy, not DVE        # d-reduce via PE transpose + ACT copy, not DVE
FP32 = mybir.dt.float32
EXP_SHIFT = -8.0             # exp(t - 8): cancels in u/Z, guards overflow


OUT_J = ROWS_PER_CORE // P   # 8 output rows per partition


def _bcast_out(t):
    """View a [P, D] tile as [P, OUT_J, D] via a step-0 middle dim."""
    return bass.AP(tensor=t.tensor, offset=t.offset, ap=[t.ap[0], [0, OUT_J], t.ap[1]])


def build_kernel(ctx, tc, h, Wa, out):
    nc = tc.nc
    const = ctx.enter_context(tc.tile_pool(name="const", bufs=1))
    hpool = ctx.enter_context(tc.tile_pool(name="hbuf", bufs=NSUP))
    prpool = ctx.enter_context(tc.tile_pool(name="prod", bufs=NSUP))
    spool = ctx.enter_context(tc.tile_pool(name="scratch", bufs=2))
    tppool = ctx.enter_context(tc.tile_pool(name="tp", bufs=NSUP))
    fpool = ctx.enter_context(tc.tile_pool(name="funnel", bufs=NSUP))
    ppool = ctx.enter_context(tc.tile_pool(name="psum", bufs=1, space="PSUM"))
    upool = ctx.enter_context(tc.tile_pool(name="upsum", bufs=1, space="PSUM"))
    tpsum = ctx.enter_context(tc.tile_pool(name="tpsum", bufs=1, space="PSUM"))

    # --- constants.  One host-concatenated tile Wa = [W | a_r^col | W.T]
    # (layout only) via SWDGE; the whole w chain is ONE matmul: stationary
    # a_col broadcast across 128 columns (stride-0 free dim) against W gives
    # w_t replicated on every partition, plus one PSUM->SBUF copy.
    Wa_sb = const.tile([D, 2 * D + 2], FP32)
    nc.gpsimd.dma_start(out=Wa_sb[:, 0 : 2 * D + 1], in_=Wa)
    Wsb = Wa_sb[:, 0:D]
    a_col = Wa_sb[:, D : D + 1]
    WT_sb = Wa_sb[:, D + 1 : 2 * D + 1]

    # scatter metadata + descriptor prep (gen early; trigger fires at the end)
    out_sb = spool.tile([P, OUT_J, D], FP32)

    junk_ps = upool.tile([1, 1], FP32)
    ones_col = const.tile([P, 1], FP32)
    nc.vector.memset(ones_col, 1.0)
    bias_sh = const.tile([P, 1], FP32)
    nc.vector.memset(bias_sh, EXP_SHIFT)

    acol_bc = bass.AP(tensor=Wa_sb.tensor, offset=a_col.offset,
                      ap=[a_col.ap[0], [0, P]])
    wb_ps = ppool.tile([P, D], FP32)
    pre_w = nc.tensor.matmul(
        junk_ps, lhsT=Wa_sb[:, 0:1], rhs=Wa_sb[:, 0:1], start=True, stop=True
    )
    wb_mm = nc.tensor.matmul(wb_ps, lhsT=acol_bc, rhs=Wsb, start=True, stop=True)
    add_dep_helper(wb_mm.ins, pre_w.ins, sync=False, reason="pe sees Wa dma first")
    w_base = const.tile([P, D], FP32)
    nc.vector.tensor_copy(w_base, wb_ps)

    iota2d = const.tile([P, P], mybir.dt.int32)
    nc.gpsimd.iota(iota2d, pattern=[[1, P]], base=0, channel_multiplier=-1)
    ident = const.tile([P, P], mybir.dt.bfloat16)
    nc.vector.tensor_scalar(out=ident, in0=iota2d, scalar1=0, scalar2=None,
                            op0=mybir.AluOpType.is_equal)
    onesb_col = const.tile([P, 1], mybir.dt.bfloat16)
    nc.vector.memset(onesb_col, 1.0)
    u_ps = upool.tile([D, 1], FP32)
    z_ps = upool.tile([D, 1], FP32)
    ones_zbc = bass.AP(tensor=ones_col.tensor, offset=ones_col.offset,
                       ap=[ones_col.ap[0], [0, D]])

    # h viewed as [s][p][j][d], row = 1024*s + 8*p + j: fully linear DMA
    hview = h.rearrange("(s p j) d -> s p j d", j=SUP, p=P)

    prev_mul = {}
    for s in range(NSUP):
        # s=0 streams in two half-DMAs so the first mult starts ~a half-
        # transfer earlier; the whole DVE chain (which sets the steady-state
        # floor) slides left with it.
        nhalf = 2 if s in (0, NSUP - 1) else 1  # split first (early start) and last (short tail)
        jh = SUP // nhalf
        halves = []
        for hh in range(nhalf):
            h_h = hpool.tile([P, jh, D], FP32, tag="hbuf")
            nc.sync.dma_start(out=h_h, in_=hview[s][:, hh * jh : (hh + 1) * jh, :])
            halves.append(h_h)
        # t-path: prod = h * w_t (broadcast), then segmented sum over d.
        # Order-only dep edges chain each engine's mults in arrival order.
        mul_eng = nc.vector if s in DVE_MULT_TILES else nc.gpsimd
        t_sup = tppool.tile([P, SUP], FP32)
        t_srcs = []
        for hh, h_h in enumerate(halves):
            # Only each engine's FIRST mult needs a funnel: it waits on ACT
            # (w_base), with the funnel absorbing the DMA wait.  Later mults
            # have already observed the ACT tick, so their single wait slot
            # takes the DMA directly and the funnel is unnecessary.
            if mul_eng not in prev_mul:
                jd = fpool.tile([1, 1], FP32)
                fun = mul_eng.tensor_copy(jd, h_h[0:1, 0, 0:1])
                pin = fun
            else:
                fun = None
                pin = prev_mul[mul_eng]
            hybrid = s in HYBRID_TILES
            pdt = mybir.dt.bfloat16 if hybrid else FP32
            prod = prpool.tile([P, jh * D], pdt, tag="prod")
            prod3 = bass.AP(tensor=prod.tensor, offset=prod.offset,
                            ap=[prod.ap[0], [D, jh], [1, D]])
            wsrc = bass.AP(
                tensor=w_base.tensor, offset=w_base.offset,
                ap=[w_base.ap[0], [0, jh], w_base.ap[1]],
            )
            mul = mul_eng.tensor_mul(prod3, h_h, wsrc)
            add_dep_helper(mul.ins, pin.ins, sync=False, reason="mult stream order")
            prev_mul[mul_eng] = mul
            if hybrid:
                # PE transposes prod (bf16, 1 cyc/row) into PSUM; ACT copies
                # it back; per-j ones-matmuls reduce d (now on partitions)
                nfree = jh * D
                nblk = nfree // P
                pT_ps = tpsum.tile([P, nfree], mybir.dt.bfloat16, tag="pT")
                for b in range(nblk):
                    nc.tensor.transpose(
                        pT_ps[:, b * P : (b + 1) * P],
                        prod[:, b * P : (b + 1) * P],
                        ident,
                    )
                pT_sb = prpool.tile([P, nfree], mybir.dt.bfloat16, tag="pTsb")
                nc.scalar.copy(pT_sb, pT_ps)
                t_hyb = tpsum.tile([P, jh], FP32, tag="thyb")
                for j in range(jh):
                    fo = j * D
                    b, po = fo // P, fo % P
                    nc.tensor.matmul(
                        t_hyb[:, j : j + 1],
                        lhsT=pT_sb[po : po + D, b * P : (b + 1) * P],
                        rhs=onesb_col[po : po + D],
                        start=True, stop=True, skip_group_check=True,
                    )
                t_srcs.append((hh * jh, t_hyb))
            else:
                nc.vector.tensor_reduce(
                    out=t_sup[:, hh * jh : (hh + 1) * jh],
                    in_=prod3,
                    axis=mybir.AxisListType.X,
                    op=mybir.AluOpType.add,
                )
        p_sup = tppool.tile([P, SUP], FP32)
        if t_srcs:
            for (joff, th) in t_srcs:
                jn = th.shape[1]
                nc.scalar.activation(
                    out=p_sup[:, joff : joff + jn], in_=th,
                    func=mybir.ActivationFunctionType.Exp,
                    bias=bias_sh, scale=1.0,
                )
        elif nhalf > 1 and s == NSUP - 1:
            for hh2 in range(nhalf):
                nc.scalar.activation(
                    out=p_sup[:, hh2 * jh : (hh2 + 1) * jh],
                    in_=t_sup[:, hh2 * jh : (hh2 + 1) * jh],
                    func=mybir.ActivationFunctionType.Exp,
                    bias=bias_sh, scale=1.0,
                )
        else:
            nc.scalar.activation(
                out=p_sup, in_=t_sup,
                func=mybir.ActivationFunctionType.Exp,
                bias=bias_sh, scale=1.0,
            )
        # PE prefetch touching each h tile: absorbs the DMA wait so the
        # first real u-matmul on that tile carries only the ACT wait.
        pres = [
            nc.tensor.matmul(
                junk_ps, lhsT=hb[:, 0, 0:1], rhs=hb[:, 0, 0:1], start=True, stop=True
            )
            for hb in halves
        ]
        for j in range(SUP):
            k = s * SUP + j
            hb = halves[j // jh]
            mm = nc.tensor.matmul(
                u_ps,
                lhsT=hb[:, j % jh, :],
                rhs=p_sup[:, j : j + 1],
                start=(k == 0),
                stop=(k == NT - 1),
                skip_group_check=True,
            )
            if j % jh == 0:
                add_dep_helper(
                    mm.ins, pres[j // jh].ins, sync=False, reason="pe sees h dma"
                )
        for j in range(SUP):
            nc.tensor.matmul(
                z_ps,
                lhsT=ones_zbc,
                rhs=p_sup[:, j : j + 1],
                start=(s == 0 and j == 0),
                stop=(s == NSUP - 1 and j == SUP - 1),
                skip_group_check=True,
            )

    # --- finale: 1/Z on u's partitions, ybc = (u/Z broadcast) @ W.T,
    # one [128, 64] scaled row copy, out-DMA replicates it 8x per partition
    # via a stride-0 source read (descriptor-gen starts ~1.5 us earlier than
    # materializing [128, 8, 64] first).
    rec64 = spool.tile([D, 1], FP32)
    nc.vector.reciprocal(rec64, z_ps)
    u_sc = spool.tile([D, 1], FP32)
    nc.vector.tensor_mul(u_sc, u_ps, rec64)
    ubc = bass.AP(tensor=u_sc.tensor, offset=u_sc.offset,
                  ap=[u_sc.ap[0], [0, P]])
    ybc_ps = ppool.tile([P, D], FP32)
    nc.tensor.matmul(ybc_ps, lhsT=ubc, rhs=WT_sb, start=True, stop=True)
    y2_sb = spool.tile([P, 2, D], FP32)
    ybc2 = bass.AP(tensor=ybc_ps.tensor, offset=ybc_ps.offset,
                   ap=[ybc_ps.ap[0], [0, 2], ybc_ps.ap[1]])
    nc.vector.tensor_copy(y2_sb, ybc2)
    ysrc = bass.AP(tensor=y2_sb.tensor, offset=y2_sb.offset,
                   ap=[y2_sb.ap[0], [0, OUT_J // 2], [1, 2 * D]])
    nc.sync.dma_start(
        out=out.rearrange("(p j) d -> p j d", j=OUT_J // 2), in_=ysrc
    )


def build_bass():
    nc = bacc.Bacc("TRN2", debug=False, target_bir_lowering=False)
    h = nc.dram_tensor("h", [N, D], FP32, kind="ExternalInput").ap()
    Wa = nc.dram_tensor("Wa", [D, 2 * D + 1], FP32, kind="ExternalInput").ap()
    out = nc.dram_tensor("out", [ROWS_PER_CORE, D], FP32, kind="ExternalOutput").ap()
    with tile.TileContext(nc) as tc:
        with ExitStack() as ctx:
            build_kernel(ctx, tc, h, Wa, out)
    nc.compile()
    return nc


_NC_CACHE = None


def _get_nc():
    global _NC_CACHE
    if _NC_CACHE is None:
        _NC_CACHE = build_bass()
    return _NC_CACHE


def kernel(**inputs) -> np.ndarray:
    h = np.ascontiguousarray(np.asarray(inputs["h"], dtype=np.float32))
    W = np.ascontiguousarray(np.asarray(inputs["W"], dtype=np.float32))
    aw = np.ascontiguousarray(np.asarray(inputs["attn_w"], dtype=np.float32))
    assert h.shape == (N, D) and W.shape == (D, D) and aw.shape == (1, 2 * D)

    Wa = np.ascontiguousarray(
        np.concatenate([W, aw[0, D : 2 * D][:, None], W.T], axis=1)
    )
    nc = _get_nc()
    in_map = {"h": h, "Wa": Wa}
    in_maps = [in_map for _ in range(NCORES)]
    res = bass_utils.run_bass_kernel_spmd(nc, in_maps, list(range(NCORES)))
    blocks = [res.results[i]["out"] for i in range(NCORES)]
    return np.concatenate(blocks, axis=0)


if __name__ == "__main__":
    nc = _get_nc()
    print("Bass program built OK")

